# revision 17
# baseline (speedup 1.0000x reference)
"""Trainium2 Bass kernel for the pairwise concordance-index loss.

reference:
    loss = sum_{i<j, f_i=f_j=1} relu((p_i-p_j)(t_i-t_j)) / 100 / n_pairs

Math:
  M[i,j] = f_i f_j (p_i-p_j)(t_i-t_j) = A^T B, rank 4:
      A = [f*u, f, f*p, f*t],  B = [f, f*u, -f*t, -f*p],  u = p*t
  (flags fold in because relu(f_i f_j x) = f_i f_j relu(x) for 0/1 flags)
  sum relu(M) = 0.5*(sum M + sum |M|); sum M has an O(B) closed form done
  on the host in fp64; sum |M| is the O(B^2) part done on device.

Device decomposition (8 cores, identical program, data-sharded):
  64 row-blocks of 128 rows; core k owns blocks 8k..8k+7 as two gangs of
  4. Each block processes cyclic column-offsets e=0..32 (cols 128a+128e
  mod 8192): e=1..31 at weight 1; e=0 / e=32 at weight 0.5 via
  0.5-pre-scaled slab appendices (host-side), so all device sums have
  uniform weight.

Device structure per gang (4 row-blocks in lockstep):
  K=4 bf16 matmuls generate M. The 4 blocks' matmuls are packed into
  disjoint 32-row PE groups via tile_position (rows 0/32/64/96) and run
  CONCURRENTLY (~3x PE throughput; K=4 matmuls never warm the HAM clock,
  so concurrency is the only lever). Each "quad" (4 concurrent N<=512
  matmuls) fills the 4 banks of one [128, 4, 512] PSUM tile, which is
  consumed by ONE abs-row-sum job on either the DVE
  (tensor_reduce(apply_absolute_value, axis=XY)) or the ScalarE
  (activation(Abs, accum_out)), alternating to balance both engines.
"""

import numpy as np

B = 8192
P = 128
NCORE = 8
ABLK = 8            # row-blocks per core (2 gangs of 4)
BMAIN = 5120        # main slab: 128*(7 + 33)
BCOLS = BMAIN + 2 * ABLK * P
E0_OFF = BMAIN                 # 0.5*cols[128a ..+128) at E0_OFF+128a
E32_OFF = BMAIN + ABLK * P     # 0.5*cols[128a+4096 ..+128) at E32_OFF+128a

# per gang: Q1..Q7 (N=512 quads), Q8 (N=384 quad), Q9 (e32|e0 two N=128 quads)
NJOBS = 36          # 18 two-bank tiles per gang x 2 gangs

_cache = {}


def _build():
    """Build + compile the Bass module (once per process)."""
    import concourse.bacc as bacc
    import concourse.tile as tile
    import concourse.mybir as mybir

    f32 = mybir.dt.float32
    bf16 = mybir.dt.bfloat16
    nc = bacc.Bacc("TRN2", target_bir_lowering=False, debug=False, num_devices=NCORE)

    a_dram = nc.dram_tensor("a_rows", [P, 2 * P], bf16, kind="ExternalInput")
    # 4 independent DRAM copies of the B-factor rows: each replica DMA reads
    # its own region so the 4 concurrent queues don't contend on HBM rows
    b_dram = nc.dram_tensor("b_cols", [4, 4, BCOLS], bf16, kind="ExternalInput")
    # bootstrap: A factors of gang0's 4 blocks ([4,512]) + slab cols 128..1152
    # ([4,1024]); lands ~3us before the full replicas so the first pair can
    # run as sequential unpacked matmuls and the reducers start early
    boot_dram = nc.dram_tensor("ab_boot", [4, 1536], bf16, kind="ExternalInput")
    acc_dram = nc.dram_tensor("acc", [P, NJOBS], f32, kind="ExternalOutput")

    with tile.TileContext(nc) as tc:
        with (
            tc.tile_pool(name="inp", bufs=1) as inp_pool,
            tc.tile_pool(name="accp", bufs=1) as acc_pool,
            tc.tile_pool(name="ps", bufs=4, space="PSUM") as ps,
        ):
            a_sb = inp_pool.tile([P, 2 * P], bf16)
            # bootstrap data first: tiny (12KB), lands well before the replicas
            boot_sb = inp_pool.tile([4, 1536], bf16)
            nc.sync.dma_start(boot_sb[:, :], boot_dram.ap()[:, :])
            # a_rows via gpsimd SWDGE: its queue is free at body start, so the
            # dispatch overlaps the HWDGE queues' b dispatches
            nc.gpsimd.dma_start(a_sb[:, :], a_dram.ap()[:, :])
            # replicate the 4 B-factor rows into all four 32-row groups from
            # 4 independent DRAM copies, 2 DMAs per HWDGE queue
            b_sb = inp_pool.tile([P, BCOLS], bf16)
            for q, eng in ((0, nc.sync), (2, nc.scalar), (1, nc.sync), (3, nc.scalar)):
                eng.dma_start(
                    b_sb[32 * q:32 * q + 4, 0:BCOLS], b_dram.ap()[q, :, 0:BCOLS]
                )

            acc_sb = acc_pool.tile([P, NJOBS], f32)

            job = 0
            for g in range(2):          # gangs: row-blocks 4g..4g+3
                def pair(poff, coff_of_a, n, engines, g=g):
                    """One quad split across two 2-bank tiles + their reduces.
                    engines: (engine for tile A [q0,q1], engine for tile B)."""
                    nonlocal job
                    tiles = (
                        ps.tile([P, 2, 512], f32, tag="q", name=f"qa{job}"),
                        ps.tile([P, 2, 512], f32, tag="q", name=f"qb{job}"),
                    )
                    for q in range(4):
                        coff = coff_of_a(4 * g + q)
                        nc.tensor.matmul(
                            tiles[q // 2][:, q % 2, poff:poff + n],
                            a_sb[32 * q:32 * q + 4, P * g:P * g + P],
                            b_sb[32 * q:32 * q + 4, coff:coff + n],
                            start=True,
                            stop=True,
                            tile_position=(32 * q, 0),
                        )
                    return tiles

                def reduce_tile(red, use_dve):
                    nonlocal job
                    if use_dve:
                        nc.vector.tensor_reduce(
                            acc_sb[:, job:job + 1], red,
                            axis=mybir.AxisListType.XY, op=mybir.AluOpType.add,
                            apply_absolute_value=True,
                        )
                    else:
                        nc.scalar.activation(
                            red, red,
                            mybir.ActivationFunctionType.Abs,
                            accum_out=acc_sb[:, job:job + 1],
                        )
                    job += 1

                # Q1..Q7: e=1..28 (N=512). tileA->ACT, tileB->DVE, except the
                # last quad of each gang sends both to ACT for balance
                # (16 ACT / 20 DVE jobs ~= equal busy time on both engines).
                for s in range(7):
                    both_act = (s == 6)
                    if g == 0 and s == 0:
                        # bootstrap pair: sequential unpacked matmuls off the
                        # tiny early DMA, so reducers start ~3us before the
                        # full b replicas land
                        tA = ps.tile([P, 2, 512], f32, tag="q", name="boot_a")
                        tB = ps.tile([P, 2, 512], f32, tag="q", name="boot_b")
                        for a in range(4):
                            nc.tensor.matmul(
                                (tA, tB)[a // 2][:, a % 2, 0:512],
                                boot_sb[0:4, 128 * a:128 * a + 128],
                                boot_sb[0:4, 512 + 128 * a:512 + 128 * a + 512],
                                start=True,
                                stop=True,
                            )
                    else:
                        tA, tB = pair(0, lambda a, s=s: P * a + 128 * (1 + 4 * s),
                                      512, None)
                    reduce_tile(tA[:, :, :], use_dve=False)
                    reduce_tile(tB[:, :, :], use_dve=not both_act)
                # Q8: e29..31 (N=384) -> DVE
                tA, tB = pair(0, lambda a: P * a + 128 * 29, 384, None)
                reduce_tile(tA[:, :, 0:384], use_dve=True)
                reduce_tile(tB[:, :, 0:384], use_dve=True)
                # Q9: e32 then e0 (both N=128, pre-scaled) -> DVE
                t9 = None
                tA, tB = pair(0, lambda a: E32_OFF + P * a, 128, None)
                for q in range(4):
                    coff = E0_OFF + P * (4 * g + q)
                    nc.tensor.matmul(
                        (tA, tB)[q // 2][:, q % 2, 128:256],
                        a_sb[32 * q:32 * q + 4, P * g:P * g + P],
                        b_sb[32 * q:32 * q + 4, coff:coff + 128],
                        start=True,
                        stop=True,
                        tile_position=(32 * q, 0),
                    )
                reduce_tile(tA[:, :, 0:256], use_dve=True)
                reduce_tile(tB[:, :, 0:256], use_dve=True)

            assert job == NJOBS, job
            nc.sync.dma_start(acc_dram.ap()[:, :], acc_sb[:, :])

    nc.compile()
    return nc


def _get_nc():
    if "nc" not in _cache:
        _cache["nc"] = _build()
    return _cache["nc"]


def _make_in_maps(p, t, f, u):
    import ml_dtypes

    A = np.ascontiguousarray(
        np.stack([f * u, f, f * p, f * t]).astype(ml_dtypes.bfloat16)
    )
    Bm = np.ascontiguousarray(
        np.stack([f, f * u, -f * t, -f * p]).astype(ml_dtypes.bfloat16)
    )
    Bh = Bm * np.asarray(0.5, dtype=ml_dtypes.bfloat16)  # exact halving

    in_maps = []
    for k in range(NCORE):
        # a_rows layout: row 32q+r = factor r of row-block 4g+q, cols 128g..+128
        a_rows = np.zeros((P, 2 * P), dtype=ml_dtypes.bfloat16)
        for g in range(2):
            for q in range(4):
                a = 4 * g + q
                rows = slice(1024 * k + P * a, 1024 * k + P * a + P)
                a_rows[32 * q:32 * q + 4, P * g:P * g + P] = A[:, rows]

        b_cols = np.empty((4, BCOLS), dtype=ml_dtypes.bfloat16)
        cols = (1024 * k + np.arange(BMAIN)) % B
        b_cols[:, 0:BMAIN] = Bm[:, cols]
        e0_cols = (1024 * k + np.arange(ABLK * P)) % B
        b_cols[:, E0_OFF:E0_OFF + ABLK * P] = Bh[:, e0_cols]
        e32_cols = (1024 * k + 4096 + np.arange(ABLK * P)) % B
        b_cols[:, E32_OFF:E32_OFF + ABLK * P] = Bh[:, e32_cols]
        b_rep = np.broadcast_to(b_cols[None], (4, 4, BCOLS))
        # bootstrap: gang0 blocks' A factors + slab cols 128..1152
        ab_boot = np.empty((4, 1536), dtype=ml_dtypes.bfloat16)
        for a in range(4):
            rows = slice(1024 * k + P * a, 1024 * k + P * a + P)
            ab_boot[:, 128 * a:128 * a + 128] = A[:, rows]
        ab_boot[:, 512:1536] = b_cols[:, 128:1152]
        in_maps.append(
            {"a_rows": a_rows, "b_cols": np.ascontiguousarray(b_rep),
             "ab_boot": ab_boot}
        )
    return in_maps, A, Bm


def kernel(pred, gt, gt_fracTime, gt_ifMOF):
    from concourse import bass_utils

    pred = np.asarray(pred)
    gt = np.asarray(gt)
    ift = int(np.asarray(gt_fracTime))
    imf = int(np.asarray(gt_ifMOF))

    p = pred.astype(np.float32)
    t = gt[:, ift].astype(np.float32)
    f = (gt[:, imf] == 1).astype(np.float32)
    u = (p * t).astype(np.float32)

    in_maps, A, Bm = _make_in_maps(p, t, f, u)
    nc = _get_nc()
    res = bass_utils.run_bass_kernel_spmd(nc, in_maps, core_ids=list(range(NCORE)))

    # T = sum_{i<j} ff |M| (all device accumulator columns are weight 1)
    T = 0.0
    for r in res.results:
        T += r["acc"].astype(np.float64).sum()

    # host closed form in fp64 over the same bf16 values the device used:
    # sum_{i<j} M = (sum_{i,j} M - sum_diag M) / 2
    A64 = A.astype(np.float64)
    B64 = Bm.astype(np.float64)
    S_all = (A64.sum(axis=1) * B64.sum(axis=1)).sum()
    D_diag = (A64 * B64).sum()
    S_half = (S_all - D_diag) / 2.0

    f64 = f.astype(np.float64)
    S_f = f64.sum()
    n_pairs = (S_f * S_f - S_f) / 2.0

    loss = 0.5 * (S_half + T) / 100.0 / n_pairs
    return np.asarray(np.float32(loss))



# revision 22
# speedup vs baseline: 1.0401x; 1.0401x over previous
"""Trainium2 Bass kernel for the pairwise concordance-index loss.

reference:
    loss = sum_{i<j, f_i=f_j=1} relu((p_i-p_j)(t_i-t_j)) / 100 / n_pairs

Math:
  M[i,j] = f_i f_j (p_i-p_j)(t_i-t_j) = A^T B, rank 4:
      A = [f*u, f, f*p, f*t],  B = [f, f*u, -f*t, -f*p],  u = p*t
  (flags fold in because relu(f_i f_j x) = f_i f_j relu(x) for 0/1 flags)
  sum relu(M) = 0.5*(sum M + sum |M|); sum M has an O(B) closed form done
  on the host in fp64; sum |M| is the O(B^2) part done on device.

Device decomposition (8 cores, identical program, data-sharded):
  64 row-blocks of 128 rows; core k owns blocks 8k..8k+7 as two gangs of
  4. Each block processes cyclic column-offsets e=0..32 (cols 128a+128e
  mod 8192): e=1..31 at weight 1; e=0 / e=32 at weight 0.5 via
  0.5-pre-scaled slab appendices (host-side), so all device sums have
  uniform weight.

Device structure per gang (4 row-blocks in lockstep):
  K=4 bf16 matmuls generate M. The 4 blocks' matmuls are packed into
  disjoint 32-row PE groups via tile_position (rows 0/32/64/96) and run
  CONCURRENTLY (~3x PE throughput; K=4 matmuls never warm the HAM clock,
  so concurrency is the only lever). Each "quad" (4 concurrent N<=512
  matmuls) fills two 2-bank PSUM tiles, each consumed by ONE abs-row-sum
  job on either the DVE (tensor_reduce(apply_absolute_value, axis=XY))
  or the ScalarE (activation(Abs, accum_out)); 16 ACT / 20 DVE jobs
  equalizes busy time on both engines (the per-core wall time is bound
  by this PSUM drain at ~1 elem/cycle/partition/engine).

Input staging: a_rows goes over the gpsimd SWDGE queue; the 4 b-replica
DMAs (one per 32-row PE group) read 4 independent DRAM copies, two per
HWDGE queue (sync + scalar), so dispatches overlap and the queues don't
contend on the same HBM rows. Keeping the reducers' start AFTER all
input lands (slack-fed pipeline) measurably beats starting them early:
a tight pipeline pays ~150ns of semaphore-poll latency per reduce job.
"""

import numpy as np

B = 8192
P = 128
NCORE = 8
ABLK = 8            # row-blocks per core (2 gangs of 4)
BMAIN = 5120        # main slab: 128*(7 + 33)
BCOLS = BMAIN + 2 * ABLK * P
E0_OFF = BMAIN                 # 0.5*cols[128a ..+128) at E0_OFF+128a
E32_OFF = BMAIN + ABLK * P     # 0.5*cols[128a+4096 ..+128) at E32_OFF+128a

# per gang: Q1..Q7 (N=512 quads), Q8 (N=384 quad), Q9 (e32|e0 two N=128 quads)
NJOBS = 36          # 18 two-bank tiles per gang x 2 gangs

_cache = {}


def _build():
    """Build + compile the Bass module (once per process)."""
    import concourse.bacc as bacc
    import concourse.tile as tile
    import concourse.mybir as mybir

    f32 = mybir.dt.float32
    bf16 = mybir.dt.bfloat16
    nc = bacc.Bacc("TRN2", target_bir_lowering=False, debug=False, num_devices=NCORE)

    a_dram = nc.dram_tensor("a_rows", [P, 2 * P], bf16, kind="ExternalInput")
    # 4 independent DRAM copies of the B-factor rows: each replica DMA reads
    # its own region so the 4 concurrent queues don't contend on HBM rows
    b_dram = nc.dram_tensor("b_cols", [4, 4, BCOLS], bf16, kind="ExternalInput")
    acc_dram = nc.dram_tensor("acc", [P, NJOBS], f32, kind="ExternalOutput")

    with tile.TileContext(nc) as tc:
        with (
            tc.tile_pool(name="inp", bufs=1) as inp_pool,
            tc.tile_pool(name="accp", bufs=1) as acc_pool,
            tc.tile_pool(name="ps", bufs=4, space="PSUM") as ps,
        ):
            a_sb = inp_pool.tile([P, 2 * P], bf16)
            # a_rows via gpsimd SWDGE: its queue is free at body start, so the
            # dispatch overlaps the HWDGE queues' b dispatches
            nc.gpsimd.dma_start(a_sb[:, :], a_dram.ap()[:, :])
            # replicate the 4 B-factor rows into all four 32-row groups from
            # 4 independent DRAM copies, 2 DMAs per HWDGE queue
            b_sb = inp_pool.tile([P, BCOLS], bf16)
            for q, eng in ((0, nc.sync), (2, nc.scalar), (1, nc.sync), (3, nc.scalar)):
                eng.dma_start(
                    b_sb[32 * q:32 * q + 4, 0:BCOLS], b_dram.ap()[q, :, 0:BCOLS]
                )

            acc_sb = acc_pool.tile([P, NJOBS], f32)

            job = 0
            for g in range(2):          # gangs: row-blocks 4g..4g+3
                def pair(poff, coff_of_a, n, engines, g=g):
                    """One quad split across two 2-bank tiles + their reduces.
                    engines: (engine for tile A [q0,q1], engine for tile B)."""
                    nonlocal job
                    tiles = (
                        ps.tile([P, 2, 512], f32, tag="q", name=f"qa{job}"),
                        ps.tile([P, 2, 512], f32, tag="q", name=f"qb{job}"),
                    )
                    for q in range(4):
                        coff = coff_of_a(4 * g + q)
                        nc.tensor.matmul(
                            tiles[q // 2][:, q % 2, poff:poff + n],
                            a_sb[32 * q:32 * q + 4, P * g:P * g + P],
                            b_sb[32 * q:32 * q + 4, coff:coff + n],
                            start=True,
                            stop=True,
                            tile_position=(32 * q, 0),
                        )
                    return tiles

                def reduce_tile(red, use_dve):
                    nonlocal job
                    if use_dve:
                        nc.vector.tensor_reduce(
                            acc_sb[:, job:job + 1], red,
                            axis=mybir.AxisListType.XY, op=mybir.AluOpType.add,
                            apply_absolute_value=True,
                        )
                    else:
                        nc.scalar.activation(
                            red, red,
                            mybir.ActivationFunctionType.Abs,
                            accum_out=acc_sb[:, job:job + 1],
                        )
                    job += 1

                # Q1..Q7: e=1..28 (N=512). tileA->ACT, tileB->DVE, except the
                # last quad of each gang sends both to ACT for balance
                # (16 ACT / 20 DVE jobs ~= equal busy time on both engines).
                for s in range(7):
                    both_act = (s == 6)
                    tA, tB = pair(0, lambda a, s=s: P * a + 128 * (1 + 4 * s), 512,
                                  None)
                    reduce_tile(tA[:, :, :], use_dve=False)
                    reduce_tile(tB[:, :, :], use_dve=not both_act)
                # Q8: e29..31 (N=384) -> DVE
                tA, tB = pair(0, lambda a: P * a + 128 * 29, 384, None)
                reduce_tile(tA[:, :, 0:384], use_dve=True)
                reduce_tile(tB[:, :, 0:384], use_dve=True)
                # Q9: e32 then e0 (both N=128, pre-scaled) -> DVE
                t9 = None
                tA, tB = pair(0, lambda a: E32_OFF + P * a, 128, None)
                for q in range(4):
                    coff = E0_OFF + P * (4 * g + q)
                    nc.tensor.matmul(
                        (tA, tB)[q // 2][:, q % 2, 128:256],
                        a_sb[32 * q:32 * q + 4, P * g:P * g + P],
                        b_sb[32 * q:32 * q + 4, coff:coff + 128],
                        start=True,
                        stop=True,
                        tile_position=(32 * q, 0),
                    )
                reduce_tile(tA[:, :, 0:256], use_dve=True)
                reduce_tile(tB[:, :, 0:256], use_dve=True)

            assert job == NJOBS, job
            nc.sync.dma_start(acc_dram.ap()[:, :], acc_sb[:, :])

    nc.compile()
    return nc


def _get_nc():
    if "nc" not in _cache:
        _cache["nc"] = _build()
    return _cache["nc"]


def _make_in_maps(p, t, f, u):
    import ml_dtypes

    A = np.ascontiguousarray(
        np.stack([f * u, f, f * p, f * t]).astype(ml_dtypes.bfloat16)
    )
    Bm = np.ascontiguousarray(
        np.stack([f, f * u, -f * t, -f * p]).astype(ml_dtypes.bfloat16)
    )
    Bh = Bm * np.asarray(0.5, dtype=ml_dtypes.bfloat16)  # exact halving

    in_maps = []
    for k in range(NCORE):
        # a_rows layout: row 32q+r = factor r of row-block 4g+q, cols 128g..+128
        a_rows = np.zeros((P, 2 * P), dtype=ml_dtypes.bfloat16)
        for g in range(2):
            for q in range(4):
                a = 4 * g + q
                rows = slice(1024 * k + P * a, 1024 * k + P * a + P)
                a_rows[32 * q:32 * q + 4, P * g:P * g + P] = A[:, rows]

        b_cols = np.empty((4, BCOLS), dtype=ml_dtypes.bfloat16)
        cols = (1024 * k + np.arange(BMAIN)) % B
        b_cols[:, 0:BMAIN] = Bm[:, cols]
        e0_cols = (1024 * k + np.arange(ABLK * P)) % B
        b_cols[:, E0_OFF:E0_OFF + ABLK * P] = Bh[:, e0_cols]
        e32_cols = (1024 * k + 4096 + np.arange(ABLK * P)) % B
        b_cols[:, E32_OFF:E32_OFF + ABLK * P] = Bh[:, e32_cols]
        b_rep = np.broadcast_to(b_cols[None], (4, 4, BCOLS))
        in_maps.append(
            {"a_rows": a_rows, "b_cols": np.ascontiguousarray(b_rep)}
        )
    return in_maps, A, Bm


def kernel(pred, gt, gt_fracTime, gt_ifMOF):
    from concourse import bass_utils

    pred = np.asarray(pred)
    gt = np.asarray(gt)
    ift = int(np.asarray(gt_fracTime))
    imf = int(np.asarray(gt_ifMOF))

    p = pred.astype(np.float32)
    t = gt[:, ift].astype(np.float32)
    f = (gt[:, imf] == 1).astype(np.float32)
    u = (p * t).astype(np.float32)

    in_maps, A, Bm = _make_in_maps(p, t, f, u)
    nc = _get_nc()
    res = bass_utils.run_bass_kernel_spmd(nc, in_maps, core_ids=list(range(NCORE)))

    # T = sum_{i<j} ff |M| (all device accumulator columns are weight 1)
    T = 0.0
    for r in res.results:
        T += r["acc"].astype(np.float64).sum()

    # host closed form in fp64 over the same bf16 values the device used:
    # sum_{i<j} M = (sum_{i,j} M - sum_diag M) / 2
    A64 = A.astype(np.float64)
    B64 = Bm.astype(np.float64)
    S_all = (A64.sum(axis=1) * B64.sum(axis=1)).sum()
    D_diag = (A64 * B64).sum()
    S_half = (S_all - D_diag) / 2.0

    f64 = f.astype(np.float64)
    S_f = f64.sum()
    n_pairs = (S_f * S_f - S_f) / 2.0

    loss = 0.5 * (S_half + T) / 100.0 / n_pairs
    return np.asarray(np.float32(loss))

